# revision 43
# baseline (speedup 1.0000x reference)
"""CVRP decoder kernel for Trainium2 (8 NeuronCores, batch-data-parallel).

Computes, per batch b (B=64, P=64, N=1000, H=128):
    q_graph   = mean_n(emb) @ Wq_graph
    q_first   = encoded_q1 @ Wq_first
    q_last    = emb[last_node] @ Wq_last
    q_visited = (vis01 @ emb / N) @ W_visited          (vis01 = isneginf(mask))
    final_q   = sum of the above + load*W_load + b_load
    score     = final_q @ emb^T / sqrt(H) - dists[last_node] / sqrt(2)
    probs     = softmax(10*tanh(score) + (-BIG if visited))

Sharding: batch dim across the 8 cores (pure data parallel), 8 batches per
core processed as 4 pairs of 2 batches stacked on the 128 SBUF partitions.

Host staging (layout/dtype only; the dist gather, all matmuls and the softmax
run on device): per pair all dense inputs are byte-packed into ONE mega row
(embT bf16 | emb fp8 chunked | visited^T fp8 (+ones col -> mean rides the
visited matmul) | visited fp8 | eq1T bf16 | last-node-embT bf16) so a single
DMA per pair loads everything; dists ship negated in fp16 and one indirect
gather fetches all four pairs' distance rows.  On device the distance bias
and the -1000 mask bias are accumulated straight into the score PSUM via
identity matmuls, so the softmax chain is just tanh -> exp(+accum) ->
normalize.
"""

import json
import math
import numpy as np
from contextlib import ExitStack

import ml_dtypes

import concourse.bass as bass
import concourse.mybir as mybir
import concourse.tile as tile
from concourse.bass_utils import run_bass_kernel_spmd


def _split_excess_waits(bir_bytes: bytes, max_waits: int = 1) -> bytes:
    """Walrus in this image rejects instructions carrying too many sem waits
    ("Too many sync wait commands", e.g. on Tile's kernel-tail Drain).
    Hoist excess waits onto preceding same-engine EventSemaphore carriers
    (pure sync ops) — sems are monotonic, so a chain of instructions whose
    waits partition the original list is equivalent."""
    d = json.loads(bir_bytes)
    n = [0]
    for fn in d.get("functions", []):
        for blk in fn.get("blocks", []):
            out = []
            for ins in blk.get("instructions", []):
                si = ins.get("sync_info") or {}
                waits = si.get("on_wait") or []
                if len(waits) > max_waits:
                    extra, keep = waits[:-max_waits], waits[-max_waits:]
                    ins["sync_info"]["on_wait"] = keep
                    for i in range(0, len(extra), max_waits):
                        n[0] += 1
                        carrier = {
                            "name": f"I-waitsplit-{n[0]}",
                            "opcode": "EventSemaphore",
                            "engine": ins["engine"],
                            "ins": [],
                            "outs": [],
                            "sync_info": {
                                "on_update": [],
                                "on_wait": extra[i:i + max_waits],
                            },
                        }
                        if "debug" in ins:
                            carrier["debug"] = ins["debug"]
                        out.append(carrier)
                out.append(ins)
            blk["instructions"] = out
    return json.dumps(d).encode()


def _install_walrus_shim():
    import concourse.bass2jax as b2j
    import concourse.bass_utils as bu
    if getattr(bu, "_waitsplit_installed", False):
        return
    real = bu.compile_bir_kernel

    def patched(bir_json, tmpdir, neff_name="file.neff", **kw):
        if isinstance(bir_json, (bytes, bytearray, str)):
            if isinstance(bir_json, str):
                bir_json = bir_json.encode()
            bir_json = _split_excess_waits(bir_json)
        return real(bir_json, tmpdir, neff_name=neff_name, **kw)

    bu.compile_bir_kernel = patched
    b2j.compile_bir_kernel = patched
    bu._waitsplit_installed = True


_install_walrus_shim()

F32 = mybir.dt.float32
F16 = mybir.dt.float16
BF = mybir.dt.bfloat16
F8 = mybir.dt.float8e4
U8 = mybir.dt.uint8
I32 = mybir.dt.int32
OP = mybir.AluOpType
AF = mybir.ActivationFunctionType

NP_BF = ml_dtypes.bfloat16
NP_F8 = ml_dtypes.float8_e4m3

B, P, N, H = 64, 64, 1000, 128
NCORES = 8
NB = B // NCORES          # 8 batches per core
NPAIR = NB // 2           # 4 pairs
NC8 = 8                   # padded n-chunks of 128 (1024 rows, last 24 zero)

MASK_NEG = -1000.0        # additive bias for visited nodes (pre x10 exp scale)
FQ_SCALE = math.sqrt(2.0) / math.sqrt(H)   # = 0.125 exactly
TANH_SCALE = 1.0 / math.sqrt(2.0)
TANH_CLIP = 10.0

# qv-feed row byte layout (per pair, per partition row)
MA_E8 = 0                 # emb fp8 [2*8*128] (j,chunk,h)
MA_V8 = 2048              # visited^T+ones fp8 [8*130] (chunk, j*65+q)
MA_BYTES = 3088
# score-feed row byte layout
MB_ET = 0                 # embT fp8 [2000]   (j*1000+n)
MB_MK = 2000              # visited fp8 [1000] (row-major [p,n])
MB_Q1 = 3000              # eq1T bf16 [128]   (j*64+p)
MB_LT = 3256              # last-node embT bf16 [128] (j*64+p)
MB_BYTES = 3512

# const row byte layout
CM_IM = 0                 # bf16 diag(-1000) [128]
CM_WQ = 256               # [Wq_graph|Wq_first|Wq_last|W_visited] bf16 [512]
CM_BYTES = 1280


def build_nc():
    nc = bass.Bass()

    mega = nc.dram_tensor("mega", [NPAIR * 128, MA_BYTES], U8,
                          kind="ExternalInput")
    megb = nc.dram_tensor("megb", [NPAIR * 128, MB_BYTES], U8,
                          kind="ExternalInput")
    combo = nc.dram_tensor("combo", [NB * N, N], F16, kind="ExternalInput")
    idxt = nc.dram_tensor("idxt", [P * 2, NPAIR], I32, kind="ExternalInput")
    cmt = nc.dram_tensor("cmt", [128, CM_BYTES], U8, kind="ExternalInput")
    wst = nc.dram_tensor("wst", [1, 3 * H + NPAIR * 128], BF,
                         kind="ExternalInput")
    probs = nc.dram_tensor("probs", [NB * P, N], BF, kind="ExternalOutput")

    with tile.TileContext(nc) as tc:
        with ExitStack() as ctx:
            const = ctx.enter_context(tc.tile_pool(name="const", bufs=1))
            inp = ctx.enter_context(tc.tile_pool(name="inp", bufs=4))
            work = ctx.enter_context(tc.tile_pool(name="work", bufs=2))
            ps_sc = ctx.enter_context(
                tc.tile_pool(name="ps_sc", bufs=2, space="PSUM"))
            ps_sm = ctx.enter_context(
                tc.tile_pool(name="ps_sm", bufs=2, space="PSUM"))

            # ---- tiny gather-index load first, so gathers dispatch ASAP ----
            idxa = const.tile([P * 2, NPAIR], I32, tag="idxa")
            nc.sync.dma_start(idxa[:], idxt[:])

            # ---- per-pair indirect gathers of -dist rows, dispatched early ----
            gs = []
            for pr in range(NPAIR):
                g = inp.tile([128, N], F16, tag="g", name=f"g{pr}")
                nc.gpsimd.indirect_dma_start(
                    out=g[:], out_offset=None, in_=combo[:],
                    in_offset=bass.IndirectOffsetOnAxis(
                        ap=idxa[:, pr:pr + 1], axis=0))
                gs.append(g)

            # ---- input loads, wire-ordered so pair k's score feed lands
            # ---- before pair k+1's qv feed (SP keeps emission order)
            megs = [None] * NPAIR
            megbs = [None] * NPAIR

            def issue_a(pr):
                m = inp.tile([128, MA_BYTES], U8, tag="mega", name=f"meg{pr}")
                nc.sync.dma_start(m[:], mega[128 * pr:128 * pr + 128, :])
                megs[pr] = m

            def issue_b(pr):
                m = inp.tile([128, MB_BYTES], U8, tag="megb", name=f"megb{pr}")
                nc.sync.dma_start(m[:], megb[128 * pr:128 * pr + 128, :])
                megbs[pr] = m

            issue_a(0)
            cm = const.tile([128, CM_BYTES], U8, tag="cm")
            nc.sync.dma_start(cm[:], cmt[:])
            ws = const.tile([1, 3 * H + NPAIR * 128], BF, tag="ws")
            nc.sync.dma_start(ws[:], wst[:])
            issue_b(0)
            for pr in range(1, NPAIR):
                issue_a(pr)
                issue_b(pr)

            identm = cm[:, CM_IM:CM_WQ].bitcast(BF)       # diag(-1000)
            wqv = cm[:, CM_WQ:CM_BYTES].bitcast(BF)       # [128, 512]
            wg, wf, wl, wv = (wqv[:, 128 * k:128 * (k + 1)] for k in range(4))
            wld = ws[0:1, 0:H]
            bld = ws[0:1, H:2 * H]
            ones_row = ws[0:1, 2 * H:3 * H]
            lda = ws[0:1, 3 * H:]

            for pr in range(NPAIR):
                r0 = 128 * pr
                m, mb = megs[pr], megbs[pr]
                e8 = m[:, MA_E8:MA_V8].bitcast(F8)         # [128, 2048]
                v8 = m[:, MA_V8:MA_BYTES].bitcast(F8)      # [128, 1040]
                et = mb[:, MB_ET:MB_MK].bitcast(F8)        # [128, 2000]
                mkt = mb[:, MB_MK:MB_Q1].bitcast(F8)       # [128, 1000]
                q1 = mb[:, MB_Q1:MB_LT].bitcast(BF)        # [128, 128]
                lastT = mb[:, MB_LT:MB_BYTES].bitcast(BF)  # [128, 128]

                # ---- visited-sum + mean via one fp8 matmul chain ----
                # pqv[h, 65j+q] = sum_n emb[b_j, n, h] * vis8[b_j, n, q]
                # (col 64 of each 65-block is the ones column -> N*mean)
                pqv = ps_sm.tile([128, 130], F32, tag="pqv", bufs=2)
                for c in range(NC8):
                    for j in range(2):
                        nc.tensor.matmul(
                            pqv[:, 65 * j:65 * j + 65],
                            lhsT=e8[:, 1024 * j + 128 * c:1024 * j + 128 * c + 128],
                            rhs=v8[:, 130 * c + 65 * j:130 * c + 65 * j + 65],
                            start=(c == 0), stop=(c == NC8 - 1))
                qvs = work.tile([128, 128], BF, tag="qvs")
                nc.vector.tensor_scalar(out=qvs[:, 0:64], in0=pqv[:, 0:64],
                                        scalar1=1.0 / N, scalar2=None,
                                        op0=OP.mult)
                nc.vector.tensor_scalar(out=qvs[:, 64:128], in0=pqv[:, 65:129],
                                        scalar1=1.0 / N, scalar2=None,
                                        op0=OP.mult)
                mrep = work.tile([128, 128], BF, tag="mrep")
                for j in range(2):
                    nc.vector.tensor_scalar(
                        out=mrep[:, 64 * j:64 * j + 64],
                        in0=pqv[:, 65 * j + 64:65 * j + 65].to_broadcast([128, 64]),
                        scalar1=1.0 / N, scalar2=None, op0=OP.mult)

                # ---- final_q^T accumulation: psum [h, 2p] ----
                pfq = ps_sm.tile([128, 128], F32, tag="pfq", bufs=1)
                nc.tensor.matmul(pfq[:], lhsT=wf, rhs=q1[:, :],
                                 start=True, stop=False)
                nc.tensor.matmul(pfq[:], lhsT=wl, rhs=lastT[:, :],
                                 start=False, stop=False)
                nc.tensor.matmul(pfq[:], lhsT=wg, rhs=mrep[:],
                                 start=False, stop=False)
                nc.tensor.matmul(pfq[:], lhsT=wv, rhs=qvs[:],
                                 start=False, stop=False)
                nc.tensor.matmul(pfq[:], lhsT=wld,
                                 rhs=lda[0:1, 128 * pr:128 * pr + 128],
                                 start=False, stop=False)
                nc.tensor.matmul(pfq[:], lhsT=bld, rhs=ones_row,
                                 start=False, stop=True)
                fqT = work.tile([128, 128], BF, tag="fqT")
                nc.vector.tensor_scalar(out=fqT[:], in0=pfq[:],
                                        scalar1=FQ_SCALE, scalar2=None,
                                        op0=OP.mult)

                # ---- score psum = q.e*0.125 - 1000*mask ----
                psc = ps_sc.tile([128, N], F32, tag="psc")
                for (n0, n1) in ((0, 512), (512, N)):
                    # mask bias: diag(-1000) x {0,1}
                    nc.tensor.matmul(psc[:, n0:n1], lhsT=identm[:, :],
                                     rhs=mkt[:, n0:n1], start=True, stop=False)
                    for j in range(2):
                        nc.tensor.matmul(
                            psc[64 * j:64 * j + 64, n0:n1],
                            lhsT=fqT[:, 64 * j:64 * j + 64],
                            rhs=et[:, N * j + n0:N * j + n1],
                            start=False, stop=(j == 1))

                # ---- dist bias (gathered -dists) off the PE stream ----
                u = work.tile([128, N], F32, tag="u")
                nc.vector.tensor_tensor(out=u[:], in0=psc[:], in1=gs[pr][:],
                                        op=OP.add)

                # ---- tanh / softmax ----
                t = work.tile([128, N], F32, tag="t")
                nc.scalar.activation(t[:], u[:], AF.Tanh, scale=TANH_SCALE)
                e = work.tile([128, N], F32, tag="e")
                ssum = work.tile([128, 1], F32, tag="ssum")
                nc.scalar.activation(e[:], t[:], AF.Exp, scale=TANH_CLIP,
                                     accum_out=ssum[:])
                rec = work.tile([128, 1], F32, tag="rec")
                nc.vector.reciprocal(out=rec[:], in_=ssum[:])
                pout = work.tile([128, N], BF, tag="pout")
                nc.vector.tensor_tensor(
                    out=pout[:], in0=e[:],
                    in1=rec[:, 0:1].to_broadcast([128, N]), op=OP.mult)
                nc.sync.dma_start(probs[r0:r0 + 128, :], pout[:])

    return nc


_CACHE = {}


def _get_nc():
    if "nc" not in _CACHE:
        _CACHE["nc"] = build_nc()
    return _CACHE["nc"]


def _shard_inputs(inputs):
    dists = np.asarray(inputs["dists"], dtype=np.float32)        # [B,N,N]
    emb = np.asarray(inputs["embeddings"], dtype=np.float32)     # [B,N,H]
    eq1 = np.asarray(inputs["encoded_q1"], dtype=np.float32)     # [B,P,H]
    last = np.asarray(inputs["last_node"]).astype(np.int64)      # [B,P]
    load = np.asarray(inputs["load"], dtype=np.float32)          # [B,P]
    mask = np.asarray(inputs["group_ninf_mask"], dtype=np.float32)
    vis = np.isneginf(mask)                                      # bool [B,P,N]

    # gather table: row (b,n) = -dists[b,n,:] in fp16
    combo = (-dists).astype(np.float16)                          # [B,N,N]

    # embT pair-packed bf16 view source
    embT = np.ascontiguousarray(emb.transpose(0, 2, 1))          # [B,H,N]

    # emb as fp8, n-chunked with n padded to 1024: [B, 128, chunk, h]
    embp = np.zeros((B, NC8 * 128, H), dtype=np.float32)
    embp[:, :N] = emb
    emb8 = np.ascontiguousarray(
        embp.reshape(B, NC8, 128, H).transpose(0, 2, 1, 3)).astype(NP_F8)

    # visited^T with ones column (for mean): [B, 128, chunk, 65]
    v = np.zeros((B, NC8 * 128, P + 1), dtype=np.float32)
    v[:, :N, :P] = vis.transpose(0, 2, 1)
    v[:, :, P] = 1.0
    vis8 = np.ascontiguousarray(
        v.reshape(B, NC8, 128, P + 1).transpose(0, 2, 1, 3)).astype(NP_F8)

    mk8 = vis.astype(NP_F8)                                      # [B,P,N] {0,1}
    eq1T = np.ascontiguousarray(eq1.transpose(0, 2, 1))          # [B,H,P]
    # last-node embeddings, gathered + transposed on host: [B,H,P]
    lastemb = np.take_along_axis(emb, last[:, :, None], axis=1)  # [B,P,H]
    lastT = np.ascontiguousarray(lastemb.transpose(0, 2, 1))     # [B,H,P]
    # flat row index into the per-core [NB*N, :] gather table
    idxflat = (last + (np.arange(B) % NB)[:, None] * N).astype(np.int32)

    # packed order must match wg, wf, wl, wv slices
    wq_pack = np.concatenate(
        [np.asarray(inputs["Wq_graph"], dtype=np.float32),
         np.asarray(inputs["Wq_first"], dtype=np.float32),
         np.asarray(inputs["Wq_last"], dtype=np.float32),
         np.asarray(inputs["W_visited"], dtype=np.float32)], axis=1)
    wq_pack = wq_pack.astype(NP_BF)

    in_maps = []
    u8 = np.uint8
    for c in range(NCORES):
        s = slice(c * NB, (c + 1) * NB)
        embT_c = embT[s].reshape(NPAIR, 2, H, N).transpose(0, 2, 1, 3) \
            .reshape(NPAIR * H, 2 * N).astype(NP_F8)
        emb8_c = emb8[s].reshape(NPAIR, 2, 128, NC8, 128) \
            .transpose(0, 2, 1, 3, 4).reshape(NPAIR * 128, 2 * NC8 * 128)
        vis8_c = vis8[s].reshape(NPAIR, 2, 128, NC8, P + 1) \
            .transpose(0, 2, 3, 1, 4).reshape(NPAIR * 128, NC8 * 130)
        eq1T_c = eq1T[s].reshape(NPAIR, 2, H, P).transpose(0, 2, 1, 3) \
            .reshape(NPAIR * H, 2 * P).astype(NP_BF)
        lastT_c = lastT[s].reshape(NPAIR, 2, H, P).transpose(0, 2, 1, 3) \
            .reshape(NPAIR * H, 2 * P).astype(NP_BF)
        meg = np.empty((NPAIR * 128, MA_BYTES), dtype=u8)
        meg[:, MA_E8:MA_V8] = np.ascontiguousarray(emb8_c).view(u8)
        meg[:, MA_V8:MA_BYTES] = np.ascontiguousarray(vis8_c).view(u8)
        mgb = np.empty((NPAIR * 128, MB_BYTES), dtype=u8)
        mgb[:, MB_ET:MB_MK] = np.ascontiguousarray(embT_c).view(u8)
        mgb[:, MB_MK:MB_Q1] = mk8[s].reshape(NPAIR * 128, N).view(u8)
        mgb[:, MB_Q1:MB_LT] = np.ascontiguousarray(eq1T_c).view(u8)
        mgb[:, MB_LT:MB_BYTES] = np.ascontiguousarray(lastT_c).view(u8)

        cmg = np.zeros((128, CM_BYTES), dtype=u8)
        cmg[:, CM_IM:CM_WQ] = (MASK_NEG * np.eye(128, dtype=np.float32)) \
            .astype(NP_BF).view(u8)
        cmg[:, CM_WQ:CM_BYTES] = np.ascontiguousarray(wq_pack).view(u8)

        wsml = np.concatenate(
            [np.asarray(inputs["W_load"], dtype=np.float32),
             np.asarray(inputs["b_load"], dtype=np.float32),
             np.ones(H, dtype=np.float32),
             load[s].reshape(NPAIR * 128)]
        ).reshape(1, 3 * H + NPAIR * 128).astype(NP_BF)

        in_maps.append(dict(
            mega=meg,
            megb=mgb,
            combo=combo[s].reshape(NB * N, N),
            idxt=np.ascontiguousarray(idxflat[s].reshape(NPAIR, 2 * P).T),
            cmt=cmg,
            wst=wsml,
        ))
    return in_maps


def _run(inputs, trace=False, **kw):
    nc = _get_nc()
    in_maps = _shard_inputs(inputs)
    res = run_bass_kernel_spmd(nc, in_maps, list(range(NCORES)),
                               trace=trace, **kw)
    out = np.concatenate(
        [np.asarray(r["probs"]).astype(np.float32).reshape(NB, P, N)
         for r in res.results], axis=0)
    return out, res


def kernel(**inputs) -> np.ndarray:
    out, _ = _run(inputs)
    return out


# revision 46
# speedup vs baseline: 1.0158x; 1.0158x over previous
"""CVRP decoder kernel for Trainium2 (8 NeuronCores, batch-data-parallel).

Computes, per batch b (B=64, P=64, N=1000, H=128):
    q_graph   = mean_n(emb) @ Wq_graph
    q_first   = encoded_q1 @ Wq_first
    q_last    = emb[last_node] @ Wq_last
    q_visited = (vis01 @ emb / N) @ W_visited          (vis01 = isneginf(mask))
    final_q   = sum of the above + load*W_load + b_load
    score     = final_q @ emb^T / sqrt(H) - dists[last_node] / sqrt(2)
    probs     = softmax(10*tanh(score) + (-BIG if visited))

Sharding: batch dim across the 8 cores (pure data parallel), 8 batches per
core processed as 4 pairs of 2 batches stacked on the 128 SBUF partitions.

Host staging (layout/dtype only; the dist gather, all matmuls and the softmax
run on device): per pair all dense inputs are byte-packed into ONE mega row
(embT bf16 | emb fp8 chunked | visited^T fp8 (+ones col -> mean rides the
visited matmul) | visited fp8 | eq1T bf16 | last-node-embT bf16) so a single
DMA per pair loads everything; dists ship negated in fp16 and one indirect
gather fetches all four pairs' distance rows.  On device the distance bias
and the -1000 mask bias are accumulated straight into the score PSUM via
identity matmuls, so the softmax chain is just tanh -> exp(+accum) ->
normalize.
"""

import json
import math
import numpy as np
from contextlib import ExitStack

import ml_dtypes

import concourse.bass as bass
import concourse.mybir as mybir
import concourse.tile as tile
from concourse.bass_utils import run_bass_kernel_spmd


def _split_excess_waits(bir_bytes: bytes, max_waits: int = 1) -> bytes:
    """Walrus in this image rejects instructions carrying too many sem waits
    ("Too many sync wait commands", e.g. on Tile's kernel-tail Drain).
    Hoist excess waits onto preceding same-engine EventSemaphore carriers
    (pure sync ops) — sems are monotonic, so a chain of instructions whose
    waits partition the original list is equivalent."""
    d = json.loads(bir_bytes)
    n = [0]
    for fn in d.get("functions", []):
        for blk in fn.get("blocks", []):
            out = []
            for ins in blk.get("instructions", []):
                si = ins.get("sync_info") or {}
                waits = si.get("on_wait") or []
                if len(waits) > max_waits:
                    extra, keep = waits[:-max_waits], waits[-max_waits:]
                    ins["sync_info"]["on_wait"] = keep
                    for i in range(0, len(extra), max_waits):
                        n[0] += 1
                        carrier = {
                            "name": f"I-waitsplit-{n[0]}",
                            "opcode": "EventSemaphore",
                            "engine": ins["engine"],
                            "ins": [],
                            "outs": [],
                            "sync_info": {
                                "on_update": [],
                                "on_wait": extra[i:i + max_waits],
                            },
                        }
                        if "debug" in ins:
                            carrier["debug"] = ins["debug"]
                        out.append(carrier)
                out.append(ins)
            blk["instructions"] = out
    return json.dumps(d).encode()


def _install_walrus_shim():
    import concourse.bass2jax as b2j
    import concourse.bass_utils as bu
    if getattr(bu, "_waitsplit_installed", False):
        return
    real = bu.compile_bir_kernel

    def patched(bir_json, tmpdir, neff_name="file.neff", **kw):
        if isinstance(bir_json, (bytes, bytearray, str)):
            if isinstance(bir_json, str):
                bir_json = bir_json.encode()
            bir_json = _split_excess_waits(bir_json)
        return real(bir_json, tmpdir, neff_name=neff_name, **kw)

    bu.compile_bir_kernel = patched
    b2j.compile_bir_kernel = patched
    bu._waitsplit_installed = True


_install_walrus_shim()

F32 = mybir.dt.float32
F16 = mybir.dt.float16
BF = mybir.dt.bfloat16
F8 = mybir.dt.float8e4
U8 = mybir.dt.uint8
I32 = mybir.dt.int32
OP = mybir.AluOpType
AF = mybir.ActivationFunctionType

NP_BF = ml_dtypes.bfloat16
NP_F8 = ml_dtypes.float8_e4m3

B, P, N, H = 64, 64, 1000, 128
NCORES = 8
NB = B // NCORES          # 8 batches per core
NPAIR = NB // 2           # 4 pairs
NC8 = 8                   # padded n-chunks of 128 (1024 rows, last 24 zero)

MASK_NEG = -1000.0        # additive bias for visited nodes (pre x10 exp scale)
FQ_SCALE = math.sqrt(2.0) / math.sqrt(H)   # = 0.125 exactly
TANH_SCALE = 1.0 / math.sqrt(2.0)
TANH_CLIP = 10.0

# qv-feed row byte layout (per pair, per partition row)
MA_E8 = 0                 # emb fp8 [2*8*128] (j,chunk,h)
MA_V8 = 2048              # visited^T+ones fp8 [8*130] (chunk, j*65+q)
MA_BYTES = 3088
# score-feed row byte layout
MB_ET = 0                 # embT bf16 [2000]  (j*1000+n)
MB_MK = 4000              # visited fp8 [1000] (row-major [p,n])
MB_Q1 = 5000              # eq1T bf16 [128]   (j*64+p)
MB_LT = 5256              # last-node embT bf16 [128] (j*64+p)
MB_BYTES = 5512

# const row byte layout
CM_IM = 0                 # bf16 diag(-1000) [128]
CM_WQ = 256               # [Wq_graph|Wq_first|Wq_last|W_visited] bf16 [512]
CM_BYTES = 1280


def build_nc():
    nc = bass.Bass()

    mega = nc.dram_tensor("mega", [NPAIR * 128, MA_BYTES], U8,
                          kind="ExternalInput")
    megb = nc.dram_tensor("megb", [NPAIR * 128, MB_BYTES], U8,
                          kind="ExternalInput")
    combo = nc.dram_tensor("combo", [NB * N, N], F16, kind="ExternalInput")
    idxt = nc.dram_tensor("idxt", [P * 2, NPAIR], I32, kind="ExternalInput")
    cmt = nc.dram_tensor("cmt", [128, CM_BYTES], U8, kind="ExternalInput")
    wst = nc.dram_tensor("wst", [1, 3 * H + NPAIR * 128], BF,
                         kind="ExternalInput")
    probs = nc.dram_tensor("probs", [NB * P, N], BF, kind="ExternalOutput")

    with tile.TileContext(nc) as tc:
        with ExitStack() as ctx:
            const = ctx.enter_context(tc.tile_pool(name="const", bufs=1))
            inp = ctx.enter_context(tc.tile_pool(name="inp", bufs=4))
            work = ctx.enter_context(tc.tile_pool(name="work", bufs=2))
            ps_sc = ctx.enter_context(
                tc.tile_pool(name="ps_sc", bufs=2, space="PSUM"))
            ps_sm = ctx.enter_context(
                tc.tile_pool(name="ps_sm", bufs=2, space="PSUM"))

            # ---- tiny gather-index load first, so gathers dispatch ASAP ----
            idxa = const.tile([P * 2, NPAIR], I32, tag="idxa")
            nc.sync.dma_start(idxa[:], idxt[:])

            # ---- per-pair indirect gathers of -dist rows, dispatched early ----
            gs = []
            for pr in range(NPAIR):
                g = inp.tile([128, N], F16, tag="g", name=f"g{pr}")
                nc.gpsimd.indirect_dma_start(
                    out=g[:], out_offset=None, in_=combo[:],
                    in_offset=bass.IndirectOffsetOnAxis(
                        ap=idxa[:, pr:pr + 1], axis=0))
                gs.append(g)

            # ---- input loads, wire-ordered so pair k's score feed lands
            # ---- before pair k+1's qv feed (SP keeps emission order)
            megs = [None] * NPAIR
            megbs = [None] * NPAIR

            def issue_a(pr):
                m = inp.tile([128, MA_BYTES], U8, tag="mega", name=f"meg{pr}")
                nc.sync.dma_start(m[:], mega[128 * pr:128 * pr + 128, :])
                megs[pr] = m

            def issue_b(pr):
                m = inp.tile([128, MB_BYTES], U8, tag="megb", name=f"megb{pr}")
                nc.sync.dma_start(m[:], megb[128 * pr:128 * pr + 128, :])
                megbs[pr] = m

            issue_a(0)
            cm = const.tile([128, CM_BYTES], U8, tag="cm")
            nc.sync.dma_start(cm[:], cmt[:])
            ws = const.tile([1, 3 * H + NPAIR * 128], BF, tag="ws")
            nc.sync.dma_start(ws[:], wst[:])
            issue_b(0)
            for pr in range(1, NPAIR):
                issue_a(pr)
                issue_b(pr)

            identm = cm[:, CM_IM:CM_WQ].bitcast(BF)       # diag(-1000)
            wqv = cm[:, CM_WQ:CM_BYTES].bitcast(BF)       # [128, 512]
            wg, wf, wl, wv = (wqv[:, 128 * k:128 * (k + 1)] for k in range(4))
            wld = ws[0:1, 0:H]
            bld = ws[0:1, H:2 * H]
            ones_row = ws[0:1, 2 * H:3 * H]
            lda = ws[0:1, 3 * H:]

            for pr in range(NPAIR):
                r0 = 128 * pr
                m, mb = megs[pr], megbs[pr]
                e8 = m[:, MA_E8:MA_V8].bitcast(F8)         # [128, 2048]
                v8 = m[:, MA_V8:MA_BYTES].bitcast(F8)      # [128, 1040]
                et = mb[:, MB_ET:MB_MK].bitcast(BF)        # [128, 2000]
                mkt = mb[:, MB_MK:MB_Q1].bitcast(F8)       # [128, 1000]
                q1 = mb[:, MB_Q1:MB_LT].bitcast(BF)        # [128, 128]
                lastT = mb[:, MB_LT:MB_BYTES].bitcast(BF)  # [128, 128]

                # ---- visited-sum + mean via one fp8 matmul chain ----
                # pqv[h, 65j+q] = sum_n emb[b_j, n, h] * vis8[b_j, n, q]
                # (col 64 of each 65-block is the ones column -> N*mean)
                pqv = ps_sm.tile([128, 130], F32, tag="pqv", bufs=2)
                for c in range(NC8):
                    for j in range(2):
                        nc.tensor.matmul(
                            pqv[:, 65 * j:65 * j + 65],
                            lhsT=e8[:, 1024 * j + 128 * c:1024 * j + 128 * c + 128],
                            rhs=v8[:, 130 * c + 65 * j:130 * c + 65 * j + 65],
                            start=(c == 0), stop=(c == NC8 - 1))
                qvs = work.tile([128, 128], BF, tag="qvs")
                nc.vector.tensor_scalar(out=qvs[:, 0:64], in0=pqv[:, 0:64],
                                        scalar1=1.0 / N, scalar2=None,
                                        op0=OP.mult)
                nc.vector.tensor_scalar(out=qvs[:, 64:128], in0=pqv[:, 65:129],
                                        scalar1=1.0 / N, scalar2=None,
                                        op0=OP.mult)
                mrep = work.tile([128, 128], BF, tag="mrep")
                for j in range(2):
                    nc.vector.tensor_scalar(
                        out=mrep[:, 64 * j:64 * j + 64],
                        in0=pqv[:, 65 * j + 64:65 * j + 65].to_broadcast([128, 64]),
                        scalar1=1.0 / N, scalar2=None, op0=OP.mult)

                # ---- final_q^T accumulation: psum [h, 2p] ----
                pfq = ps_sm.tile([128, 128], F32, tag="pfq", bufs=1)
                nc.tensor.matmul(pfq[:], lhsT=wf, rhs=q1[:, :],
                                 start=True, stop=False)
                nc.tensor.matmul(pfq[:], lhsT=wl, rhs=lastT[:, :],
                                 start=False, stop=False)
                nc.tensor.matmul(pfq[:], lhsT=wg, rhs=mrep[:],
                                 start=False, stop=False)
                nc.tensor.matmul(pfq[:], lhsT=wv, rhs=qvs[:],
                                 start=False, stop=False)
                nc.tensor.matmul(pfq[:], lhsT=wld,
                                 rhs=lda[0:1, 128 * pr:128 * pr + 128],
                                 start=False, stop=False)
                nc.tensor.matmul(pfq[:], lhsT=bld, rhs=ones_row,
                                 start=False, stop=True)
                fqT = work.tile([128, 128], BF, tag="fqT")
                nc.vector.tensor_scalar(out=fqT[:], in0=pfq[:],
                                        scalar1=FQ_SCALE, scalar2=None,
                                        op0=OP.mult)

                # ---- score psum = q.e*0.125 - 1000*mask ----
                psc = ps_sc.tile([128, N], F32, tag="psc")
                for (n0, n1) in ((0, 512), (512, N)):
                    # mask bias: diag(-1000) x {0,1}
                    nc.tensor.matmul(psc[:, n0:n1], lhsT=identm[:, :],
                                     rhs=mkt[:, n0:n1], start=True, stop=False)
                    for j in range(2):
                        nc.tensor.matmul(
                            psc[64 * j:64 * j + 64, n0:n1],
                            lhsT=fqT[:, 64 * j:64 * j + 64],
                            rhs=et[:, N * j + n0:N * j + n1],
                            start=False, stop=(j == 1))

                # ---- dist bias (gathered -dists) off the PE stream ----
                u = work.tile([128, N], F32, tag="u")
                nc.vector.tensor_tensor(out=u[:], in0=psc[:], in1=gs[pr][:],
                                        op=OP.add)

                # ---- tanh / softmax ----
                t = work.tile([128, N], F32, tag="t")
                nc.scalar.activation(t[:], u[:], AF.Tanh, scale=TANH_SCALE)
                e = work.tile([128, N], F32, tag="e")
                ssum = work.tile([128, 1], F32, tag="ssum")
                nc.scalar.activation(e[:], t[:], AF.Exp, scale=TANH_CLIP,
                                     accum_out=ssum[:])
                rec = work.tile([128, 1], F32, tag="rec")
                nc.vector.reciprocal(out=rec[:], in_=ssum[:])
                pout = work.tile([128, N], BF, tag="pout")
                nc.vector.tensor_tensor(
                    out=pout[:], in0=e[:],
                    in1=rec[:, 0:1].to_broadcast([128, N]), op=OP.mult)
                nc.sync.dma_start(probs[r0:r0 + 128, :], pout[:])

    return nc


_CACHE = {}


def _get_nc():
    if "nc" not in _CACHE:
        _CACHE["nc"] = build_nc()
    return _CACHE["nc"]


def _shard_inputs(inputs):
    dists = np.asarray(inputs["dists"], dtype=np.float32)        # [B,N,N]
    emb = np.asarray(inputs["embeddings"], dtype=np.float32)     # [B,N,H]
    eq1 = np.asarray(inputs["encoded_q1"], dtype=np.float32)     # [B,P,H]
    last = np.asarray(inputs["last_node"]).astype(np.int64)      # [B,P]
    load = np.asarray(inputs["load"], dtype=np.float32)          # [B,P]
    mask = np.asarray(inputs["group_ninf_mask"], dtype=np.float32)
    vis = np.isneginf(mask)                                      # bool [B,P,N]

    # gather table: row (b,n) = -dists[b,n,:] in fp16
    combo = (-dists).astype(np.float16)                          # [B,N,N]

    # embT pair-packed bf16 view source
    embT = np.ascontiguousarray(emb.transpose(0, 2, 1))          # [B,H,N]

    # emb as fp8, n-chunked with n padded to 1024: [B, 128, chunk, h]
    embp = np.zeros((B, NC8 * 128, H), dtype=np.float32)
    embp[:, :N] = emb
    emb8 = np.ascontiguousarray(
        embp.reshape(B, NC8, 128, H).transpose(0, 2, 1, 3)).astype(NP_F8)

    # visited^T with ones column (for mean): [B, 128, chunk, 65]
    v = np.zeros((B, NC8 * 128, P + 1), dtype=np.float32)
    v[:, :N, :P] = vis.transpose(0, 2, 1)
    v[:, :, P] = 1.0
    vis8 = np.ascontiguousarray(
        v.reshape(B, NC8, 128, P + 1).transpose(0, 2, 1, 3)).astype(NP_F8)

    mk8 = vis.astype(NP_F8)                                      # [B,P,N] {0,1}
    eq1T = np.ascontiguousarray(eq1.transpose(0, 2, 1))          # [B,H,P]
    # last-node embeddings, gathered + transposed on host: [B,H,P]
    lastemb = np.take_along_axis(emb, last[:, :, None], axis=1)  # [B,P,H]
    lastT = np.ascontiguousarray(lastemb.transpose(0, 2, 1))     # [B,H,P]
    # flat row index into the per-core [NB*N, :] gather table
    idxflat = (last + (np.arange(B) % NB)[:, None] * N).astype(np.int32)

    # packed order must match wg, wf, wl, wv slices
    wq_pack = np.concatenate(
        [np.asarray(inputs["Wq_graph"], dtype=np.float32),
         np.asarray(inputs["Wq_first"], dtype=np.float32),
         np.asarray(inputs["Wq_last"], dtype=np.float32),
         np.asarray(inputs["W_visited"], dtype=np.float32)], axis=1)
    wq_pack = wq_pack.astype(NP_BF)

    in_maps = []
    u8 = np.uint8
    for c in range(NCORES):
        s = slice(c * NB, (c + 1) * NB)
        embT_c = embT[s].reshape(NPAIR, 2, H, N).transpose(0, 2, 1, 3) \
            .reshape(NPAIR * H, 2 * N).astype(NP_BF)
        emb8_c = emb8[s].reshape(NPAIR, 2, 128, NC8, 128) \
            .transpose(0, 2, 1, 3, 4).reshape(NPAIR * 128, 2 * NC8 * 128)
        vis8_c = vis8[s].reshape(NPAIR, 2, 128, NC8, P + 1) \
            .transpose(0, 2, 3, 1, 4).reshape(NPAIR * 128, NC8 * 130)
        eq1T_c = eq1T[s].reshape(NPAIR, 2, H, P).transpose(0, 2, 1, 3) \
            .reshape(NPAIR * H, 2 * P).astype(NP_BF)
        lastT_c = lastT[s].reshape(NPAIR, 2, H, P).transpose(0, 2, 1, 3) \
            .reshape(NPAIR * H, 2 * P).astype(NP_BF)
        meg = np.empty((NPAIR * 128, MA_BYTES), dtype=u8)
        meg[:, MA_E8:MA_V8] = np.ascontiguousarray(emb8_c).view(u8)
        meg[:, MA_V8:MA_BYTES] = np.ascontiguousarray(vis8_c).view(u8)
        mgb = np.empty((NPAIR * 128, MB_BYTES), dtype=u8)
        mgb[:, MB_ET:MB_MK] = np.ascontiguousarray(embT_c).view(u8)
        mgb[:, MB_MK:MB_Q1] = mk8[s].reshape(NPAIR * 128, N).view(u8)
        mgb[:, MB_Q1:MB_LT] = np.ascontiguousarray(eq1T_c).view(u8)
        mgb[:, MB_LT:MB_BYTES] = np.ascontiguousarray(lastT_c).view(u8)

        cmg = np.zeros((128, CM_BYTES), dtype=u8)
        cmg[:, CM_IM:CM_WQ] = (MASK_NEG * np.eye(128, dtype=np.float32)) \
            .astype(NP_BF).view(u8)
        cmg[:, CM_WQ:CM_BYTES] = np.ascontiguousarray(wq_pack).view(u8)

        wsml = np.concatenate(
            [np.asarray(inputs["W_load"], dtype=np.float32),
             np.asarray(inputs["b_load"], dtype=np.float32),
             np.ones(H, dtype=np.float32),
             load[s].reshape(NPAIR * 128)]
        ).reshape(1, 3 * H + NPAIR * 128).astype(NP_BF)

        in_maps.append(dict(
            mega=meg,
            megb=mgb,
            combo=combo[s].reshape(NB * N, N),
            idxt=np.ascontiguousarray(idxflat[s].reshape(NPAIR, 2 * P).T),
            cmt=cmg,
            wst=wsml,
        ))
    return in_maps


def _run(inputs, trace=False, **kw):
    nc = _get_nc()
    in_maps = _shard_inputs(inputs)
    res = run_bass_kernel_spmd(nc, in_maps, list(range(NCORES)),
                               trace=trace, **kw)
    out = np.concatenate(
        [np.asarray(r["probs"]).astype(np.float32).reshape(NB, P, N)
         for r in res.results], axis=0)
    return out, res


def kernel(**inputs) -> np.ndarray:
    out, _ = _run(inputs)
    return out
